# revision 19
# baseline (speedup 1.0000x reference)
"""3-layer GAT (PyG GATConv semantics, self-loops) on 8 Trainium2 NeuronCores.

Strategy (per spec sharding hint: 1D node partition, replicated weights):
  - Host: degree-balanced bin-packing of nodes into 128-node "dst tiles"
    (392 tiles = 8 cores x 49 tiles); edges land in their dst node's tile,
    padded to `cpt` chunks of 128 edge slots per tile.
  - Device, per layer L: each core computes the node table
    t_L = [h_L || e_src_L || e_dst_L] for its node shard (dense matmuls),
    AllGather -> full table; then for each dst tile: indirect-DMA gather of
    t_L rows keyed by edge src, one-hot segment matrices S/S^T on the
    tensor engine perform the ed-expand, the attention-weighted aggregation
    and the softmax denominators; normalization folds the 1/denom into a
    per-dst scalar after aggregation (segment-max subtraction is skipped --
    logits are O(1) so exp is safe, softmax is shift-invariant).
  - bf16 tables/messages, fp32 PSUM accumulation (validated: rel err ~3e-3).
"""

import os
import sys
import heapq

import numpy as np

sys.path.insert(0, "/opt/trn_rl_repo")

import ml_dtypes  # noqa: E402

from concourse import bass, bacc, mybir  # noqa: E402
import concourse.tile as tile  # noqa: E402
from concourse.bass_utils import run_bass_kernel_spmd  # noqa: E402

BF16 = mybir.dt.bfloat16
F32 = mybir.dt.float32
I32 = mybir.dt.int32
AF = mybir.ActivationFunctionType
ALU = mybir.AluOpType
NPBF = ml_dtypes.bfloat16

NEG = 0.2
EPS = 1e-20
IN, HID, HEADS, OUT = 128, 64, 8, 3
ROW1 = HEADS * HID + 2 * HEADS  # 528: h1 | es1(8) | ed1(8)
ROW2 = HID + 2                  # 66:  h2 | es2 | ed2
ROW3 = 8                        # 8:   h3(3) | es3 | ed3 | pad(3)

LAST_EXEC_NS = None


# ----------------------------------------------------------------------------
# Host preprocessing
# ----------------------------------------------------------------------------

def _pack_tiles(deg, n_tiles):
    """Greedy degree-balanced packing of nodes into n_tiles tiles of <=128."""
    n = len(deg)
    order = np.argsort(-deg, kind="stable")
    heap = [(0, t) for t in range(n_tiles)]
    heapq.heapify(heap)
    counts = np.zeros(n_tiles, np.int64)
    sums = np.zeros(n_tiles, np.int64)
    tile_of = np.empty(n, np.int64)
    slot_of = np.empty(n, np.int64)
    for node in order:
        d = int(deg[node])
        s, t = heapq.heappop(heap)
        tile_of[node] = t
        slot_of[node] = counts[t]
        counts[t] += 1
        sums[t] = s + d
        if counts[t] < 128:
            heapq.heappush(heap, (s + d, t))
    return tile_of, slot_of, sums


def _fold_attn_weights(W, a_s, a_d):
    W = np.asarray(W, np.float64)
    a_s = np.asarray(a_s, np.float64)
    a_d = np.asarray(a_d, np.float64)
    H, C = a_s.shape
    Ws = np.stack([W[:, h * C:(h + 1) * C] @ a_s[h] for h in range(H)], 1)
    Wd = np.stack([W[:, h * C:(h + 1) * C] @ a_d[h] for h in range(H)], 1)
    return Ws.astype(np.float32), Wd.astype(np.float32)


def _prepare(inputs, n_cores, t_pc, tpg):
    x = np.asarray(inputs["x"], np.float32)
    ei = np.asarray(inputs["edge_idx"], np.int64)
    n = x.shape[0]
    n_tiles = n_cores * t_pc
    n_pad = n_tiles * 128
    assert n_pad >= n, (n_pad, n)

    # self-loops are handled on-device via the local diagonal path; only
    # real edges go through the gather/segment machinery
    src = ei[0]
    dst = ei[1]

    deg = np.bincount(dst, minlength=n)
    tile_of, slot_of, sums = _pack_tiles(deg, n_tiles)
    cpt = max(4, int(-(-sums.max() // 128)))  # chunks (of 128 edges) per tile

    pos = tile_of * 128 + slot_of  # node -> global padded slot
    # chunked-AllGather row layout: tables are gathered in ag_t-tile chunks,
    # each chunk concatenating all cores' blocks, so the gather index of a
    # node differs from `pos`
    ag_t = 7 if t_pc % 7 == 0 else t_pc
    n_ag = t_pc // ag_t
    c_of = tile_of // t_pc
    t_of = tile_of % t_pc
    k_of = t_of // ag_t
    r_of = t_of % ag_t
    pos_g = (k_of * (n_cores * ag_t * 128)
             + (c_of * ag_t + r_of) * 128 + slot_of)
    # permuted x, transposed per core: xT[c] = [128feat, shard]
    shard = t_pc * 128
    x_perm = np.zeros((n_pad, IN), np.float32)
    x_perm[pos] = x

    # edges in permuted space, grouped by dst tile
    e_dst_pos = pos[dst]
    e_tile = e_dst_pos >> 7
    e_dl = e_dst_pos & 127
    e_src_pos = pos_g[src]
    order = np.argsort(e_tile, kind="stable")
    e_tile = e_tile[order]
    e_dl = e_dl[order]
    e_src_pos = e_src_pos[order]
    tile_start = np.searchsorted(e_tile, np.arange(n_tiles))
    tile_end = np.searchsorted(e_tile, np.arange(n_tiles) + 1)

    slots = cpt * 128
    idx_all = np.zeros((n_tiles, slots), np.int32)
    dl_all = np.full((n_tiles, slots), 999.0, np.float32)
    for t in range(n_tiles):
        a, b = tile_start[t], tile_end[t]
        ne = b - a
        assert ne <= slots
        idx_all[t, :ne] = e_src_pos[a:b]
        dl_all[t, :ne] = e_dl[a:b]

    # group layout: groups of `tpg` tiles -> [n_groups, 128, tpg*cpt] arrays
    n_groups = -(-t_pc // tpg)
    gch = tpg * cpt  # chunks per (full) group

    # weights
    W1 = np.asarray(inputs["W1"], np.float32)
    Ws1, Wd1 = _fold_attn_weights(W1, inputs["as1"], inputs["ad1"])
    W2 = np.asarray(inputs["W2"], np.float32)
    Ws2, Wd2 = _fold_attn_weights(W2, inputs["as2"], inputs["ad2"])
    W3 = np.asarray(inputs["W3"], np.float32)
    Ws3, Wd3 = _fold_attn_weights(W3, inputs["as3"], inputs["ad3"])
    b1 = np.asarray(inputs["b1"], np.float32)
    b2 = np.asarray(inputs["b2"], np.float32)
    b3 = np.asarray(inputs["b3"], np.float32)

    w2sd = np.concatenate([W2, Ws2, Wd2], 1)  # [512, 66]
    w2sd_r = np.zeros((128, 4 * ROW2), np.float32)
    for k in range(4):
        w2sd_r[:, k * ROW2:(k + 1) * ROW2] = w2sd[k * 128:(k + 1) * 128]
    w3sd = np.concatenate([W3, Ws3, Wd3], 1)  # [64, 5]

    iota = np.broadcast_to(np.arange(128, dtype=np.float32), (128, 128)).copy()
    ident = np.eye(128, dtype=np.float32)

    common = {
        "w1": W1.astype(NPBF),
        "wsd1": np.concatenate([Ws1, Wd1], 1).astype(NPBF),  # [128, 16]
        "w2sd": w2sd_r.astype(NPBF),
        "w3sd": w3sd.astype(NPBF),
        "b1b": np.broadcast_to(b1, (128, HEADS * HID)).copy(),
        "b2b": np.broadcast_to(b2, (128, HID)).copy(),
        "b3b": np.broadcast_to(b3, (128, OUT)).copy(),
        "iota": iota,
        "ident": ident.astype(NPBF),
    }

    per_core = []
    for c in range(n_cores):
        t0 = c * t_pc
        idx_c = idx_all[t0:t0 + t_pc].reshape(t_pc * cpt, 128)  # [chunks,128]
        dl_c = dl_all[t0:t0 + t_pc].reshape(t_pc * cpt, 128)
        nch_tot = t_pc * cpt
        idxg = np.zeros((n_groups, 128, gch), np.int32)
        dlg = np.full((n_groups, 128, gch), 999.0, np.float32)
        stg = np.zeros((n_groups, 128, gch * 128), NPBF)
        sg = np.zeros((n_groups, 128, gch * 128), NPBF)
        for g in range(n_groups):
            c0 = g * gch
            c1 = min(c0 + gch, nch_tot)
            m = c1 - c0
            idxg[g, :, :m] = idx_c[c0:c1].T
            dlg[g, :, :m] = dl_c[c0:c1].T
            # S^T[d, e] = (dl[e] == d); S[e, d] likewise transposed
            eq = (dl_c[c0:c1, None, :] == np.arange(128, dtype=np.float32)[None, :, None])
            eqb = eq.astype(NPBF)
            stg[g, :, :m * 128] = eqb.transpose(1, 0, 2).reshape(128, m * 128)
            sg[g, :, :m * 128] = eqb.transpose(2, 0, 1).reshape(128, m * 128)
        xT = np.ascontiguousarray(x_perm[c * shard:(c + 1) * shard].T).astype(NPBF)
        per_core.append({"xt": xT, "idxg": idxg, "dlg": dlg, "stg": stg,
                         "sg": sg, **common})

    return {
        "per_core": per_core,
        "pos": pos,
        "cpt": cpt,
        "n_groups": n_groups,
        "shard": shard,
    }


# ----------------------------------------------------------------------------
# Device program
# ----------------------------------------------------------------------------

def _build(n_cores, t_pc, cpt, tpg):
    n_groups = -(-t_pc // tpg)
    gch = tpg * cpt
    shard = t_pc * 128
    n_pad = n_cores * shard

    nc = bacc.Bacc(num_devices=n_cores)

    # --- I/O ---
    xt_in = nc.declare_dram_parameter("xt", [128, shard], BF16, isOutput=False)
    w1_in = nc.declare_dram_parameter("w1", [128, HEADS * HID], BF16, isOutput=False)
    wsd1_in = nc.declare_dram_parameter("wsd1", [128, 2 * HEADS], BF16, isOutput=False)
    w2sd_in = nc.declare_dram_parameter("w2sd", [128, 4 * ROW2], BF16, isOutput=False)
    w3sd_in = nc.declare_dram_parameter("w3sd", [HID, OUT + 2], BF16, isOutput=False)
    b1b_in = nc.declare_dram_parameter("b1b", [128, HEADS * HID], F32, isOutput=False)
    b2b_in = nc.declare_dram_parameter("b2b", [128, HID], F32, isOutput=False)
    b3b_in = nc.declare_dram_parameter("b3b", [128, OUT], F32, isOutput=False)
    iota_in = nc.declare_dram_parameter("iota", [128, 128], F32, isOutput=False)
    ident_in = nc.declare_dram_parameter("ident", [128, 128], BF16, isOutput=False)
    idxg_in = nc.declare_dram_parameter("idxg", [n_groups, 128, gch], I32, isOutput=False)
    dlg_in = nc.declare_dram_parameter("dlg", [n_groups, 128, gch], F32, isOutput=False)
    stg_in = nc.declare_dram_parameter("stg", [n_groups, 128, gch * 128], BF16, isOutput=False)
    sg_in = nc.declare_dram_parameter("sg", [n_groups, 128, gch * 128], BF16, isOutput=False)
    out_ext = nc.declare_dram_parameter("out", [shard, OUT], F32, isOutput=True)

    with tile.TileContext(nc) as tc:
        # internal DRAM
        _frees = []
        t1_loc, _f = tc.tile([shard, ROW1], BF16, space="DRAM", name="t1_loc")
        _frees.append(_f)
        t2_loc, _f = tc.tile([shard, ROW2], BF16, space="DRAM", name="t2_loc")
        _frees.append(_f)
        t3_loc, _f = tc.tile([shard, ROW3], BF16, space="DRAM", name="t3_loc")
        _frees.append(_f)
        t1_full, _f = tc.tile([n_pad, ROW1], BF16, space="DRAM", name="t1_full",
                              addr_space="Shared" if n_cores > 1 else "Local")
        _frees.append(_f)
        t2_full, _f = tc.tile([n_pad, ROW2], BF16, space="DRAM", name="t2_full",
                              addr_space="Shared" if n_cores > 1 else "Local")
        _frees.append(_f)
        t3_full, _f = tc.tile([n_pad, ROW3], BF16, space="DRAM", name="t3_full",
                              addr_space="Shared" if n_cores > 1 else "Local")
        _frees.append(_f)

        with (
            tc.tile_pool(name="const", bufs=1) as cpool,
            tc.tile_pool(name="keep", bufs=1) as kpool,
            tc.tile_pool(name="ld", bufs=4) as ldpool,
            tc.tile_pool(name="gat", bufs=16) as gatpool,
            tc.tile_pool(name="work", bufs=6) as wpool,
            tc.tile_pool(name="fin", bufs=3) as fpool,
            tc.tile_pool(name="pbig", bufs=3, space="PSUM") as pbig,
            tc.tile_pool(name="pden", bufs=1, space="PSUM") as pden,
            tc.tile_pool(name="ppe", bufs=2, space="PSUM") as ppe,
            tc.tile_pool(name="pmid", bufs=2, space="PSUM") as pmid,
        ):
            # constants
            w1 = cpool.tile([128, HEADS * HID], BF16, name="w1c")
            nc.sync.dma_start(out=w1[:], in_=w1_in[:])
            wsd1 = cpool.tile([128, 2 * HEADS], BF16, name="wsd1c")
            nc.sync.dma_start(out=wsd1[:], in_=wsd1_in[:])
            w2sd = cpool.tile([128, 4 * ROW2], BF16, name="w2sdc")
            nc.sync.dma_start(out=w2sd[:], in_=w2sd_in[:])
            w3sd = cpool.tile([HID, OUT + 2], BF16, name="w3sdc")
            nc.sync.dma_start(out=w3sd[:], in_=w3sd_in[:])
            b1b = cpool.tile([128, HEADS * HID], F32, name="b1bc")
            nc.sync.dma_start(out=b1b[:], in_=b1b_in[:])
            b2b = cpool.tile([128, HID], F32, name="b2bc")
            nc.sync.dma_start(out=b2b[:], in_=b2b_in[:])
            b3b = cpool.tile([128, OUT], F32, name="b3bc")
            nc.sync.dma_start(out=b3b[:], in_=b3b_in[:])
            iota = cpool.tile([128, 128], F32, name="iotac")
            nc.sync.dma_start(out=iota[:], in_=iota_in[:])
            ident = cpool.tile([128, 128], BF16, name="identc")
            nc.sync.dma_start(out=ident[:], in_=ident_in[:])

            esd1_keep = kpool.tile([128, t_pc * 2 * HEADS], BF16, name="esd1k")
            esd2_keep = kpool.tile([128, t_pc * 2], BF16, name="esd2k")
            esd3_keep = kpool.tile([128, t_pc * 2], BF16, name="esd3k")

            ag_t = 7 if t_pc % 7 == 0 else t_pc
            ag_rows = ag_t * 128

            def allgather_chunk(loc, full, k):
                lo = loc[k * ag_rows:(k + 1) * ag_rows, :]
                fu = full[k * n_cores * ag_rows:(k + 1) * n_cores * ag_rows, :]
                if n_cores > 1:
                    nc.gpsimd.collective_compute(
                        "AllGather",
                        ALU.bypass,
                        replica_groups=[list(range(n_cores))],
                        ins=[lo],
                        outs=[fu],
                    )
                else:
                    nc.sync.dma_start(out=fu, in_=lo)

            # ---------------- Phase A: layer-1 node table ----------------
            for t in range(t_pc):
                xt = ldpool.tile([128, 128], BF16, name="xt_t")
                nc.sync.dma_start(out=xt[:], in_=xt_in[:, t * 128:(t + 1) * 128])
                ph = pbig.tile([128, HEADS * HID], F32, name="phA", tag="acc")
                nc.tensor.matmul(ph[:], lhsT=xt[:], rhs=w1[:], start=True, stop=True)
                pe = pden.tile([128, 2 * HEADS], F32, name="peA", tag="den")
                nc.tensor.matmul(pe[:], lhsT=xt[:], rhs=wsd1[:], start=True, stop=True)
                t1t = wpool.tile([128, ROW1], BF16, name="t1t")
                nc.vector.tensor_tensor(
                    out=t1t[:, 0:HEADS * HID], in0=ph[:], in1=b1b[:], op=ALU.add)
                nc.scalar.copy(out=t1t[:, HEADS * HID:ROW1], in_=pe[:])
                nc.vector.tensor_copy(
                    out=esd1_keep[:, t * 16:(t + 1) * 16],
                    in_=t1t[:, HEADS * HID:ROW1])
                nc.sync.dma_start(out=t1_loc[t * 128:(t + 1) * 128, :], in_=t1t[:])
                if t % ag_t == ag_t - 1:
                    allgather_chunk(t1_loc, t1_full, t // ag_t)

            # ---------------- generic aggregation over one layer ----------
            def agg_layer(lnum, table_full, row, feat, nheads, finalize):
                """feat = message width; es at col feat, ed at col feat+nheads
                (for lnum==1 es block is [feat:feat+8], ed [feat+8:feat+16])."""
                for g in range(n_groups):
                    ntl = min(tpg, t_pc - g * tpg)
                    nch = ntl * cpt
                    idxt = ldpool.tile([128, gch], I32, name=f"idx{lnum}", tag="idx")
                    nc.sync.dma_start(out=idxt[:, :nch], in_=idxg_in[g, :, :nch])
                    dlt = ldpool.tile([128, gch], F32, name=f"dl{lnum}", tag="dl")
                    nc.sync.dma_start(out=dlt[:, :nch], in_=dlg_in[g, :, :nch])
                    stt = ldpool.tile([128, gch * 128], BF16, name=f"st{lnum}", tag="st")
                    nc.sync.dma_start(out=stt[:, :nch * 128], in_=stg_in[g, :, :nch * 128])
                    sgt = ldpool.tile([128, gch * 128], BF16, name=f"sg{lnum}", tag="sg")
                    nc.sync.dma_start(out=sgt[:, :nch * 128], in_=sg_in[g, :, :nch * 128])
                    for tl in range(ntl):
                        t = g * tpg + tl
                        if lnum == 1:
                            po = pbig.tile([128, feat], F32, name="po1", tag="acc")
                            pd = pden.tile([128, nheads], F32, name="pd1", tag="den")
                            pt = wpool.tile([128, nheads], BF16, name="pt1")
                        else:
                            po = pbig.tile([128, feat + 1], F32, name=f"po{lnum}", tag="acc")
                        for ch in range(cpt):
                            lc = tl * cpt + ch
                            first, last = ch == 0, ch == cpt - 1
                            gt = gatpool.tile([128, row], BF16, name=f"g{lnum}",
                                              tag=f"g{lnum}")
                            nc.gpsimd.indirect_dma_start(
                                out=gt[:, :],
                                out_offset=None,
                                in_=table_full[:, :],
                                in_offset=bass.IndirectOffsetOnAxis(
                                    ap=idxt[:, lc:lc + 1], axis=0),
                            )
                            S = sgt[:, lc * 128:(lc + 1) * 128]
                            pE = ppe.tile([128, nheads], F32, name=f"pE{lnum}", tag="pE")
                            if lnum == 1:
                                edt = esd1_keep[:, t * 16 + 8:t * 16 + 16]
                            elif lnum == 2:
                                edt = esd2_keep[:, t * 2 + 1:t * 2 + 2]
                            else:
                                edt = esd3_keep[:, t * 2 + 1:t * 2 + 2]
                            nc.tensor.matmul(
                                pE[:], lhsT=stt[:, lc * 128:(lc + 1) * 128],
                                rhs=edt, start=True, stop=True)
                            Ee = wpool.tile([128, nheads], F32, name=f"Ee{lnum}", tag="Ee")
                            nc.vector.tensor_tensor(
                                out=Ee[:], in0=gt[:, feat:feat + nheads],
                                in1=pE[:], op=ALU.add)
                            lr = wpool.tile([128, nheads], F32, name=f"lr{lnum}", tag="lr")
                            nc.vector.scalar_tensor_tensor(
                                out=lr[:], in0=Ee[:], scalar=NEG, in1=Ee[:],
                                op0=ALU.mult, op1=ALU.max)
                            if lnum == 1:
                                nc.scalar.activation(out=pt[:], in_=lr[:], func=AF.Exp)
                                Mw = wpool.tile([128, feat], BF16, name="Mw1")
                                nc.vector.tensor_tensor(
                                    out=Mw[:].rearrange("p (h c) -> p h c", h=HEADS),
                                    in0=gt[:, 0:feat].rearrange(
                                        "p (h c) -> p h c", h=HEADS),
                                    in1=pt[:, :, None].to_broadcast([128, HEADS, HID]),
                                    op=ALU.mult)
                                nc.tensor.matmul(po[:], lhsT=S, rhs=Mw[:],
                                                 start=first, stop=last)
                                nc.tensor.matmul(pd[:], lhsT=S, rhs=pt[:],
                                                 start=first, stop=last)
                            else:
                                Mw = wpool.tile([128, feat + 1], BF16, name=f"Mw{lnum}", tag="Mw23")
                                ptf = wpool.tile([128, 1], F32, name=f"ptf{lnum}", tag="ptf")
                                nc.scalar.activation(
                                    out=ptf[:], in_=lr[:], func=AF.Exp)
                                nc.scalar.copy(
                                    out=Mw[:, feat:feat + 1], in_=ptf[:])
                                nc.vector.tensor_scalar(
                                    out=Mw[:, 0:feat], in0=gt[:, 0:feat],
                                    scalar1=ptf[:], scalar2=None,
                                    op0=ALU.mult)
                                nc.tensor.matmul(po[:], lhsT=S, rhs=Mw[:],
                                                 start=first, stop=last)
                        if lnum == 1:
                            finalize(t, po, pd)
                        else:
                            finalize(t, po, None)

            # ---------------- Layer 1 ----------------
            def fin1(t, po, pd):
                # self-loop (local, no gather): p_self*h_tile added to po/pd
                hs = fpool.tile([128, HEADS * HID], BF16, name="hs1")
                nc.sync.dma_start(
                    out=hs[:], in_=t1_loc[t * 128:(t + 1) * 128, 0:HEADS * HID])
                Es = fpool.tile([128, HEADS], F32, name="Es1")
                nc.vector.tensor_tensor(
                    out=Es[:], in0=esd1_keep[:, t * 16:t * 16 + 8],
                    in1=esd1_keep[:, t * 16 + 8:t * 16 + 16], op=ALU.add)
                lrs = fpool.tile([128, HEADS], F32, name="lrs1")
                nc.vector.scalar_tensor_tensor(
                    out=lrs[:], in0=Es[:], scalar=NEG, in1=Es[:],
                    op0=ALU.mult, op1=ALU.max)
                psf = fpool.tile([128, HEADS], F32, name="psf1")
                nc.scalar.activation(out=psf[:], in_=lrs[:], func=AF.Exp)
                den = fpool.tile([128, HEADS], F32, name="den1")
                nc.vector.tensor_tensor(
                    out=den[:], in0=pd[:], in1=psf[:], op=ALU.add)
                rec = fpool.tile([128, HEADS], F32, name="rec1")
                nc.vector.reciprocal(rec[:], den[:])
                acc = fpool.tile([128, HEADS * HID], F32, name="acc1")
                nc.vector.tensor_tensor(
                    out=acc[:].rearrange("p (h c) -> p h c", h=HEADS),
                    in0=hs[:].rearrange("p (h c) -> p h c", h=HEADS),
                    in1=psf[:, :, None].to_broadcast([128, HEADS, HID]),
                    op=ALU.mult)
                nc.vector.tensor_tensor(
                    out=acc[:], in0=acc[:], in1=po[:], op=ALU.add)
                x1f = fpool.tile([128, HEADS * HID], F32, name="x1f")
                nc.vector.tensor_tensor(
                    out=x1f[:].rearrange("p (h c) -> p h c", h=HEADS),
                    in0=acc[:].rearrange("p (h c) -> p h c", h=HEADS),
                    in1=rec[:, :, None].to_broadcast([128, HEADS, HID]),
                    op=ALU.mult)
                x1t = fpool.tile([128, HEADS * HID], BF16, name="x1t")
                nc.scalar.activation(out=x1t[:], in_=x1f[:], func=AF.Relu)
                # layer-2 table rows
                x1T = fpool.tile([128, HEADS * HID], BF16, name="x1T")
                for k in range(4):
                    tp = pmid.tile([128, 128], BF16, name="tpT", tag="mid")
                    nc.tensor.transpose(
                        out=tp[:], in_=x1t[:, k * 128:(k + 1) * 128],
                        identity=ident[:])
                    nc.scalar.copy(out=x1T[:, k * 128:(k + 1) * 128], in_=tp[:])
                pt2 = pmid.tile([128, ROW2], F32, name="pt2", tag="mid")
                for k in range(4):
                    nc.tensor.matmul(
                        pt2[:], lhsT=x1T[:, k * 128:(k + 1) * 128],
                        rhs=w2sd[:, k * ROW2:(k + 1) * ROW2],
                        start=(k == 0), stop=(k == 3))
                t2t = fpool.tile([128, ROW2], BF16, name="t2t")
                nc.vector.tensor_tensor(
                    out=t2t[:, 0:HID], in0=pt2[:, 0:HID], in1=b2b[:], op=ALU.add)
                nc.scalar.copy(out=t2t[:, HID:ROW2], in_=pt2[:, HID:ROW2])
                nc.vector.tensor_copy(
                    out=esd2_keep[:, t * 2:(t + 1) * 2], in_=t2t[:, HID:ROW2])
                nc.sync.dma_start(out=t2_loc[t * 128:(t + 1) * 128, :], in_=t2t[:])
                if t % ag_t == ag_t - 1:
                    allgather_chunk(t2_loc, t2_full, t // ag_t)

            agg_layer(1, t1_full, ROW1, HEADS * HID, HEADS, fin1)

            # ---------------- Layer 2 ----------------
            def fin2(t, po, _):
                hs = fpool.tile([128, HID], BF16, name="hs2")
                nc.sync.dma_start(
                    out=hs[:], in_=t2_loc[t * 128:(t + 1) * 128, 0:HID])
                Es = fpool.tile([128, 1], F32, name="Es2")
                nc.vector.tensor_tensor(
                    out=Es[:], in0=esd2_keep[:, t * 2:t * 2 + 1],
                    in1=esd2_keep[:, t * 2 + 1:t * 2 + 2], op=ALU.add)
                lrs = fpool.tile([128, 1], F32, name="lrs2")
                nc.vector.scalar_tensor_tensor(
                    out=lrs[:], in0=Es[:], scalar=NEG, in1=Es[:],
                    op0=ALU.mult, op1=ALU.max)
                psf = fpool.tile([128, 1], F32, name="psf2")
                nc.scalar.activation(out=psf[:], in_=lrs[:], func=AF.Exp)
                den = fpool.tile([128, 1], F32, name="den2")
                nc.vector.tensor_tensor(
                    out=den[:], in0=po[:, HID:HID + 1], in1=psf[:], op=ALU.add)
                rec = fpool.tile([128, 1], F32, name="rec2")
                nc.vector.reciprocal(rec[:], den[:])
                acc = fpool.tile([128, HID], F32, name="acc2")
                nc.vector.tensor_scalar(
                    out=acc[:], in0=hs[:], scalar1=psf[:], scalar2=None,
                    op0=ALU.mult)
                nc.vector.tensor_tensor(
                    out=acc[:], in0=acc[:], in1=po[:, 0:HID], op=ALU.add)
                x2f = fpool.tile([128, HID], F32, name="x2f")
                nc.vector.tensor_scalar(
                    out=x2f[:], in0=acc[:], scalar1=rec[:, 0:1],
                    scalar2=None, op0=ALU.mult)
                x2t = fpool.tile([128, HID], BF16, name="x2t")
                nc.scalar.activation(out=x2t[:], in_=x2f[:], func=AF.Relu)
                tp = pmid.tile([128, 128], BF16, name="tpT2", tag="mid")
                nc.tensor.transpose(out=tp[:HID, :], in_=x2t[:], identity=ident[:])
                x2T = fpool.tile([HID, 128], BF16, name="x2T")
                nc.scalar.copy(out=x2T[:], in_=tp[:HID, :])
                pt3 = pmid.tile([128, OUT + 2], F32, name="pt3", tag="mid")
                nc.tensor.matmul(pt3[:], lhsT=x2T[:], rhs=w3sd[:],
                                 start=True, stop=True)
                t3t = fpool.tile([128, ROW3], BF16, name="t3t")
                nc.vector.tensor_tensor(
                    out=t3t[:, 0:OUT], in0=pt3[:, 0:OUT], in1=b3b[:], op=ALU.add)
                nc.scalar.copy(out=t3t[:, OUT:OUT + 2], in_=pt3[:, OUT:OUT + 2])
                nc.vector.memset(t3t[:, OUT + 2:ROW3], 0)
                nc.vector.tensor_copy(
                    out=esd3_keep[:, t * 2:(t + 1) * 2], in_=t3t[:, OUT:OUT + 2])
                nc.sync.dma_start(out=t3_loc[t * 128:(t + 1) * 128, :], in_=t3t[:])
                if t % ag_t == ag_t - 1:
                    allgather_chunk(t3_loc, t3_full, t // ag_t)

            agg_layer(2, t2_full, ROW2, HID, 1, fin2)

            # ---------------- Layer 3 ----------------
            def fin3(t, po, _):
                hs = fpool.tile([128, OUT], BF16, name="hs3")
                nc.sync.dma_start(
                    out=hs[:], in_=t3_loc[t * 128:(t + 1) * 128, 0:OUT])
                Es = fpool.tile([128, 1], F32, name="Es3")
                nc.vector.tensor_tensor(
                    out=Es[:], in0=esd3_keep[:, t * 2:t * 2 + 1],
                    in1=esd3_keep[:, t * 2 + 1:t * 2 + 2], op=ALU.add)
                lrs = fpool.tile([128, 1], F32, name="lrs3")
                nc.vector.scalar_tensor_tensor(
                    out=lrs[:], in0=Es[:], scalar=NEG, in1=Es[:],
                    op0=ALU.mult, op1=ALU.max)
                psf = fpool.tile([128, 1], F32, name="psf3")
                nc.scalar.activation(out=psf[:], in_=lrs[:], func=AF.Exp)
                den = fpool.tile([128, 1], F32, name="den3")
                nc.vector.tensor_tensor(
                    out=den[:], in0=po[:, OUT:OUT + 1], in1=psf[:], op=ALU.add)
                rec = fpool.tile([128, 1], F32, name="rec3")
                nc.vector.reciprocal(rec[:], den[:])
                acc = fpool.tile([128, OUT], F32, name="acc3")
                nc.vector.tensor_scalar(
                    out=acc[:], in0=hs[:], scalar1=psf[:], scalar2=None,
                    op0=ALU.mult)
                nc.vector.tensor_tensor(
                    out=acc[:], in0=acc[:], in1=po[:, 0:OUT], op=ALU.add)
                of = fpool.tile([128, OUT], F32, name="of")
                nc.vector.tensor_scalar(
                    out=of[:], in0=acc[:], scalar1=rec[:, 0:1],
                    scalar2=None, op0=ALU.mult)
                nc.sync.dma_start(out=out_ext[t * 128:(t + 1) * 128, :], in_=of[:])

            agg_layer(3, t3_full, ROW3, OUT, 1, fin3)

        for _f in reversed(_frees):
            _f()

    nc.compile()
    return nc


# ----------------------------------------------------------------------------
# Entry points
# ----------------------------------------------------------------------------

def _install_trace_hook():
    """Provide antenv.axon_hooks (missing in this image) so that
    run_bass_kernel_spmd(trace=True) can reach the axon NTFF profiler."""
    if "antenv.axon_hooks" in sys.modules:
        return
    import types
    import ctypes
    import contextlib

    so_path = "/opt/axon/libaxon_pjrt.so"
    mod = types.ModuleType("antenv.axon_hooks")
    state = {}

    def set_axon_ntff_profile_hook(h):
        state["h"] = h

    def get_axon_ntff_profile_hook():
        return state.get("h")

    mod.set_axon_ntff_profile_hook = set_axon_ntff_profile_hook
    mod.get_axon_ntff_profile_hook = get_axon_ntff_profile_hook
    sys.modules["antenv.axon_hooks"] = mod
    try:
        import antenv

        antenv.axon_hooks = mod
    except ImportError:
        pass

    try:
        lib = ctypes.CDLL(so_path)
    except OSError:
        return
    if not hasattr(lib, "axon_start_nrt_profile"):
        return
    lib.axon_start_nrt_profile.argtypes = [
        ctypes.POINTER(ctypes.c_int64), ctypes.c_size_t]
    lib.axon_start_nrt_profile.restype = ctypes.c_int64
    lib.axon_stop_nrt_profile.argtypes = [ctypes.c_char_p]
    lib.axon_stop_nrt_profile.restype = ctypes.c_int64

    @contextlib.contextmanager
    def _hook(output_dir, device_ids):
        import jax

        jax.devices()
        if device_ids:
            ids = (ctypes.c_int64 * len(device_ids))(*device_ids)
            rc = lib.axon_start_nrt_profile(ids, len(device_ids))
        else:
            rc = lib.axon_start_nrt_profile(None, 0)
        if rc != 0:
            raise RuntimeError(f"axon_start_nrt_profile rc={rc}")
        try:
            yield
        finally:
            n = lib.axon_stop_nrt_profile(str(output_dir).encode())
            if n < 0:
                raise RuntimeError(f"axon_stop_nrt_profile rc={n}")

    set_axon_ntff_profile_hook(_hook)


def _run(inputs, n_cores=8, t_pc=49, tpg=4, trace=False):
    global LAST_EXEC_NS
    if trace:
        _install_trace_hook()
    prep = _prepare(inputs, n_cores, t_pc, tpg)
    nc = _build(n_cores, t_pc, prep["cpt"], tpg)
    in_maps = prep["per_core"]
    res = run_bass_kernel_spmd(nc, in_maps, list(range(n_cores)), trace=trace)
    LAST_EXEC_NS = res.exec_time_ns
    outs = np.concatenate([res.results[c]["out"] for c in range(n_cores)], 0)
    n = np.asarray(inputs["x"]).shape[0]
    return np.ascontiguousarray(outs[prep["pos"][:n]]).astype(np.float32)


def kernel(**inputs):
    trace = os.environ.get("GAT_TRACE", "0") == "1"
    return _run(inputs, trace=trace)


# revision 20
# speedup vs baseline: 1.1233x; 1.1233x over previous
"""3-layer GAT (PyG GATConv semantics, self-loops) on 8 Trainium2 NeuronCores.

Strategy (per spec sharding hint: 1D node partition, replicated weights):
  - Host: degree-balanced bin-packing of nodes into 128-node "dst tiles"
    (392 tiles = 8 cores x 49 tiles); edges land in their dst node's tile,
    padded to `cpt` chunks of 128 edge slots per tile.
  - Device, per layer L: each core computes the node table
    t_L = [h_L || e_src_L || e_dst_L] for its node shard (dense matmuls),
    AllGather -> full table; then for each dst tile: indirect-DMA gather of
    t_L rows keyed by edge src, one-hot segment matrices S/S^T on the
    tensor engine perform the ed-expand, the attention-weighted aggregation
    and the softmax denominators; normalization folds the 1/denom into a
    per-dst scalar after aggregation (segment-max subtraction is skipped --
    logits are O(1) so exp is safe, softmax is shift-invariant).
  - bf16 tables/messages, fp32 PSUM accumulation (validated: rel err ~3e-3).
"""

import os
import sys
import heapq

import numpy as np

sys.path.insert(0, "/opt/trn_rl_repo")

import ml_dtypes  # noqa: E402

from concourse import bass, bacc, mybir  # noqa: E402
import concourse.tile as tile  # noqa: E402
from concourse.bass_utils import run_bass_kernel_spmd  # noqa: E402

BF16 = mybir.dt.bfloat16
F32 = mybir.dt.float32
I32 = mybir.dt.int32
AF = mybir.ActivationFunctionType
ALU = mybir.AluOpType
NPBF = ml_dtypes.bfloat16

NEG = 0.2
EPS = 1e-20
IN, HID, HEADS, OUT = 128, 64, 8, 3
ROW1 = HEADS * HID + 2 * HEADS  # 528: h1 | es1(8) | ed1(8)
ROW2 = HID + 2                  # 66:  h2 | es2 | ed2
ROW3 = 8                        # 8:   h3(3) | es3 | ed3 | pad(3)

LAST_EXEC_NS = None


# ----------------------------------------------------------------------------
# Host preprocessing
# ----------------------------------------------------------------------------

def _pack_tiles(deg, n_tiles):
    """Greedy degree-balanced packing of nodes into n_tiles tiles of <=128."""
    n = len(deg)
    order = np.argsort(-deg, kind="stable")
    heap = [(0, t) for t in range(n_tiles)]
    heapq.heapify(heap)
    counts = np.zeros(n_tiles, np.int64)
    sums = np.zeros(n_tiles, np.int64)
    tile_of = np.empty(n, np.int64)
    slot_of = np.empty(n, np.int64)
    for node in order:
        d = int(deg[node])
        s, t = heapq.heappop(heap)
        tile_of[node] = t
        slot_of[node] = counts[t]
        counts[t] += 1
        sums[t] = s + d
        if counts[t] < 128:
            heapq.heappush(heap, (s + d, t))
    return tile_of, slot_of, sums


def _fold_attn_weights(W, a_s, a_d):
    W = np.asarray(W, np.float64)
    a_s = np.asarray(a_s, np.float64)
    a_d = np.asarray(a_d, np.float64)
    H, C = a_s.shape
    Ws = np.stack([W[:, h * C:(h + 1) * C] @ a_s[h] for h in range(H)], 1)
    Wd = np.stack([W[:, h * C:(h + 1) * C] @ a_d[h] for h in range(H)], 1)
    return Ws.astype(np.float32), Wd.astype(np.float32)


def _prepare(inputs, n_cores, t_pc, tpg):
    x = np.asarray(inputs["x"], np.float32)
    ei = np.asarray(inputs["edge_idx"], np.int64)
    n = x.shape[0]
    n_tiles = n_cores * t_pc
    n_pad = n_tiles * 128
    assert n_pad >= n, (n_pad, n)

    # self-loops are handled on-device via the local diagonal path; only
    # real edges go through the gather/segment machinery
    src = ei[0]
    dst = ei[1]

    deg = np.bincount(dst, minlength=n)
    tile_of, slot_of, sums = _pack_tiles(deg, n_tiles)
    cpt = max(4, int(-(-sums.max() // 128)))  # chunks (of 128 edges) per tile

    pos = tile_of * 128 + slot_of  # node -> global padded slot
    # chunked-AllGather row layout: tables are gathered in ag_t-tile chunks,
    # each chunk concatenating all cores' blocks, so the gather index of a
    # node differs from `pos`
    ag_t = 7 if t_pc % 7 == 0 else t_pc
    n_ag = t_pc // ag_t
    c_of = tile_of // t_pc
    t_of = tile_of % t_pc
    k_of = t_of // ag_t
    r_of = t_of % ag_t
    pos_g = (k_of * (n_cores * ag_t * 128)
             + (c_of * ag_t + r_of) * 128 + slot_of)
    # permuted x, transposed per core: xT[c] = [128feat, shard]
    shard = t_pc * 128
    x_perm = np.zeros((n_pad, IN), np.float32)
    x_perm[pos] = x

    # edges in permuted space, grouped by dst tile
    e_dst_pos = pos[dst]
    e_tile = e_dst_pos >> 7
    e_dl = e_dst_pos & 127
    e_src_pos = pos_g[src]
    order = np.argsort(e_tile, kind="stable")
    e_tile = e_tile[order]
    e_dl = e_dl[order]
    e_src_pos = e_src_pos[order]
    tile_start = np.searchsorted(e_tile, np.arange(n_tiles))
    tile_end = np.searchsorted(e_tile, np.arange(n_tiles) + 1)

    slots = cpt * 128
    idx_all = np.zeros((n_tiles, slots), np.int32)
    dl_all = np.full((n_tiles, slots), 999.0, np.float32)
    for t in range(n_tiles):
        a, b = tile_start[t], tile_end[t]
        ne = b - a
        assert ne <= slots
        idx_all[t, :ne] = e_src_pos[a:b]
        dl_all[t, :ne] = e_dl[a:b]

    # group layout: groups of `tpg` tiles -> [n_groups, 128, tpg*cpt] arrays
    n_groups = -(-t_pc // tpg)
    gch = tpg * cpt  # chunks per (full) group

    # weights
    W1 = np.asarray(inputs["W1"], np.float32)
    Ws1, Wd1 = _fold_attn_weights(W1, inputs["as1"], inputs["ad1"])
    W2 = np.asarray(inputs["W2"], np.float32)
    Ws2, Wd2 = _fold_attn_weights(W2, inputs["as2"], inputs["ad2"])
    W3 = np.asarray(inputs["W3"], np.float32)
    Ws3, Wd3 = _fold_attn_weights(W3, inputs["as3"], inputs["ad3"])
    b1 = np.asarray(inputs["b1"], np.float32)
    b2 = np.asarray(inputs["b2"], np.float32)
    b3 = np.asarray(inputs["b3"], np.float32)

    w2sd = np.concatenate([W2, Ws2, Wd2], 1)  # [512, 66]
    w2sd_r = np.zeros((128, 4 * ROW2), np.float32)
    for k in range(4):
        w2sd_r[:, k * ROW2:(k + 1) * ROW2] = w2sd[k * 128:(k + 1) * 128]
    w3sd = np.concatenate([W3, Ws3, Wd3], 1)  # [64, 5]

    iota = np.broadcast_to(np.arange(128, dtype=np.float32), (128, 128)).copy()
    ident = np.eye(128, dtype=np.float32)

    common = {
        "w1": W1.astype(NPBF),
        "wsd1": np.concatenate([Ws1, Wd1], 1).astype(NPBF),  # [128, 16]
        "w2sd": w2sd_r.astype(NPBF),
        "w3sd": w3sd.astype(NPBF),
        "b1b": np.broadcast_to(b1, (128, HEADS * HID)).copy(),
        "b2b": np.broadcast_to(b2, (128, HID)).copy(),
        "b3b": np.broadcast_to(b3, (128, OUT)).copy(),
        "iota": iota,
        "ident": ident.astype(NPBF),
    }

    per_core = []
    for c in range(n_cores):
        t0 = c * t_pc
        idx_c = idx_all[t0:t0 + t_pc].reshape(t_pc * cpt, 128)  # [chunks,128]
        dl_c = dl_all[t0:t0 + t_pc].reshape(t_pc * cpt, 128)
        nch_tot = t_pc * cpt
        idxg = np.zeros((n_groups, 128, gch), np.int32)
        dlg = np.full((n_groups, 128, gch), 999.0, np.float32)
        stg = np.zeros((n_groups, 128, gch * 128), NPBF)
        sg = np.zeros((n_groups, 128, gch * 128), NPBF)
        for g in range(n_groups):
            c0 = g * gch
            c1 = min(c0 + gch, nch_tot)
            m = c1 - c0
            idxg[g, :, :m] = idx_c[c0:c1].T
            dlg[g, :, :m] = dl_c[c0:c1].T
            # S^T[d, e] = (dl[e] == d); S[e, d] likewise transposed
            eq = (dl_c[c0:c1, None, :] == np.arange(128, dtype=np.float32)[None, :, None])
            eqb = eq.astype(NPBF)
            stg[g, :, :m * 128] = eqb.transpose(1, 0, 2).reshape(128, m * 128)
            sg[g, :, :m * 128] = eqb.transpose(2, 0, 1).reshape(128, m * 128)
        xT = np.ascontiguousarray(x_perm[c * shard:(c + 1) * shard].T).astype(NPBF)
        per_core.append({"xt": xT, "idxg": idxg, "dlg": dlg, "stg": stg,
                         "sg": sg, **common})

    return {
        "per_core": per_core,
        "pos": pos,
        "cpt": cpt,
        "n_groups": n_groups,
        "shard": shard,
    }


# ----------------------------------------------------------------------------
# Device program
# ----------------------------------------------------------------------------

def _build(n_cores, t_pc, cpt, tpg):
    n_groups = -(-t_pc // tpg)
    gch = tpg * cpt
    shard = t_pc * 128
    n_pad = n_cores * shard

    nc = bacc.Bacc(num_devices=n_cores)

    # --- I/O ---
    xt_in = nc.declare_dram_parameter("xt", [128, shard], BF16, isOutput=False)
    w1_in = nc.declare_dram_parameter("w1", [128, HEADS * HID], BF16, isOutput=False)
    wsd1_in = nc.declare_dram_parameter("wsd1", [128, 2 * HEADS], BF16, isOutput=False)
    w2sd_in = nc.declare_dram_parameter("w2sd", [128, 4 * ROW2], BF16, isOutput=False)
    w3sd_in = nc.declare_dram_parameter("w3sd", [HID, OUT + 2], BF16, isOutput=False)
    b1b_in = nc.declare_dram_parameter("b1b", [128, HEADS * HID], F32, isOutput=False)
    b2b_in = nc.declare_dram_parameter("b2b", [128, HID], F32, isOutput=False)
    b3b_in = nc.declare_dram_parameter("b3b", [128, OUT], F32, isOutput=False)
    iota_in = nc.declare_dram_parameter("iota", [128, 128], F32, isOutput=False)
    ident_in = nc.declare_dram_parameter("ident", [128, 128], BF16, isOutput=False)
    idxg_in = nc.declare_dram_parameter("idxg", [n_groups, 128, gch], I32, isOutput=False)
    dlg_in = nc.declare_dram_parameter("dlg", [n_groups, 128, gch], F32, isOutput=False)
    stg_in = nc.declare_dram_parameter("stg", [n_groups, 128, gch * 128], BF16, isOutput=False)
    sg_in = nc.declare_dram_parameter("sg", [n_groups, 128, gch * 128], BF16, isOutput=False)
    out_ext = nc.declare_dram_parameter("out", [shard, OUT], F32, isOutput=True)

    with tile.TileContext(nc) as tc:
        # internal DRAM
        _frees = []
        t1_loc, _f = tc.tile([shard, ROW1], BF16, space="DRAM", name="t1_loc")
        _frees.append(_f)
        t2_loc, _f = tc.tile([shard, ROW2], BF16, space="DRAM", name="t2_loc")
        _frees.append(_f)
        t3_loc, _f = tc.tile([shard, ROW3], BF16, space="DRAM", name="t3_loc")
        _frees.append(_f)
        t1_full, _f = tc.tile([n_pad, ROW1], BF16, space="DRAM", name="t1_full",
                              addr_space="Shared" if n_cores > 1 else "Local")
        _frees.append(_f)
        t2_full, _f = tc.tile([n_pad, ROW2], BF16, space="DRAM", name="t2_full",
                              addr_space="Shared" if n_cores > 1 else "Local")
        _frees.append(_f)
        t3_full, _f = tc.tile([n_pad, ROW3], BF16, space="DRAM", name="t3_full",
                              addr_space="Shared" if n_cores > 1 else "Local")
        _frees.append(_f)

        with (
            tc.tile_pool(name="const", bufs=1) as cpool,
            tc.tile_pool(name="keep", bufs=1) as kpool,
            tc.tile_pool(name="ld", bufs=4) as ldpool,
            tc.tile_pool(name="gat", bufs=10) as gatpool,
            tc.tile_pool(name="work", bufs=6) as wpool,
            tc.tile_pool(name="fin", bufs=3) as fpool,
            tc.tile_pool(name="pbig", bufs=2, space="PSUM") as pbig,
            tc.tile_pool(name="pden", bufs=2, space="PSUM") as pden,
            tc.tile_pool(name="ppe", bufs=2, space="PSUM") as ppe,
            tc.tile_pool(name="pmid", bufs=2, space="PSUM") as pmid,
        ):
            # constants
            w1 = cpool.tile([128, HEADS * HID], BF16, name="w1c")
            nc.sync.dma_start(out=w1[:], in_=w1_in[:])
            wsd1 = cpool.tile([128, 2 * HEADS], BF16, name="wsd1c")
            nc.sync.dma_start(out=wsd1[:], in_=wsd1_in[:])
            w2sd = cpool.tile([128, 4 * ROW2], BF16, name="w2sdc")
            nc.sync.dma_start(out=w2sd[:], in_=w2sd_in[:])
            w3sd = cpool.tile([HID, OUT + 2], BF16, name="w3sdc")
            nc.sync.dma_start(out=w3sd[:], in_=w3sd_in[:])
            b1b = cpool.tile([128, HEADS * HID], F32, name="b1bc")
            nc.sync.dma_start(out=b1b[:], in_=b1b_in[:])
            b2b = cpool.tile([128, HID], F32, name="b2bc")
            nc.sync.dma_start(out=b2b[:], in_=b2b_in[:])
            b3b = cpool.tile([128, OUT], F32, name="b3bc")
            nc.sync.dma_start(out=b3b[:], in_=b3b_in[:])
            iota = cpool.tile([128, 128], F32, name="iotac")
            nc.sync.dma_start(out=iota[:], in_=iota_in[:])
            ident = cpool.tile([128, 128], BF16, name="identc")
            nc.sync.dma_start(out=ident[:], in_=ident_in[:])

            esd1_keep = kpool.tile([128, t_pc * 2 * HEADS], BF16, name="esd1k")
            esd2_keep = kpool.tile([128, t_pc * 2], BF16, name="esd2k")
            esd3_keep = kpool.tile([128, t_pc * 2], BF16, name="esd3k")

            ag_t = 7 if t_pc % 7 == 0 else t_pc
            ag_rows = ag_t * 128

            def allgather_chunk(loc, full, k):
                lo = loc[k * ag_rows:(k + 1) * ag_rows, :]
                fu = full[k * n_cores * ag_rows:(k + 1) * n_cores * ag_rows, :]
                if n_cores > 1:
                    nc.gpsimd.collective_compute(
                        "AllGather",
                        ALU.bypass,
                        replica_groups=[list(range(n_cores))],
                        ins=[lo],
                        outs=[fu],
                    )
                else:
                    nc.sync.dma_start(out=fu, in_=lo)

            # ---------------- Phase A: layer-1 node table ----------------
            for t in range(t_pc):
                xt = ldpool.tile([128, 128], BF16, name="xt_t")
                nc.sync.dma_start(out=xt[:], in_=xt_in[:, t * 128:(t + 1) * 128])
                ph = pbig.tile([128, HEADS * HID], F32, name="phA", tag="acc")
                nc.tensor.matmul(ph[:], lhsT=xt[:], rhs=w1[:], start=True, stop=True)
                pe = pden.tile([128, 2 * HEADS], F32, name="peA", tag="den")
                nc.tensor.matmul(pe[:], lhsT=xt[:], rhs=wsd1[:], start=True, stop=True)
                t1t = wpool.tile([128, ROW1], BF16, name="t1t")
                nc.vector.tensor_tensor(
                    out=t1t[:, 0:HEADS * HID], in0=ph[:], in1=b1b[:], op=ALU.add)
                nc.scalar.copy(out=t1t[:, HEADS * HID:ROW1], in_=pe[:])
                nc.vector.tensor_copy(
                    out=esd1_keep[:, t * 16:(t + 1) * 16],
                    in_=t1t[:, HEADS * HID:ROW1])
                nc.sync.dma_start(out=t1_loc[t * 128:(t + 1) * 128, :], in_=t1t[:])
                if t % ag_t == ag_t - 1:
                    allgather_chunk(t1_loc, t1_full, t // ag_t)

            # ---------------- generic aggregation over one layer ----------
            def agg_layer(lnum, table_full, row, feat, nheads, finalize):
                """feat = message width; es at col feat, ed at col feat+nheads
                (for lnum==1 es block is [feat:feat+8], ed [feat+8:feat+16])."""
                for g in range(n_groups):
                    ntl = min(tpg, t_pc - g * tpg)
                    nch = ntl * cpt
                    idxt = ldpool.tile([128, gch], I32, name=f"idx{lnum}", tag="idx")
                    nc.sync.dma_start(out=idxt[:, :nch], in_=idxg_in[g, :, :nch])
                    dlt = ldpool.tile([128, gch], F32, name=f"dl{lnum}", tag="dl")
                    nc.sync.dma_start(out=dlt[:, :nch], in_=dlg_in[g, :, :nch])
                    stt = ldpool.tile([128, gch * 128], BF16, name=f"st{lnum}", tag="st")
                    nc.sync.dma_start(out=stt[:, :nch * 128], in_=stg_in[g, :, :nch * 128])
                    sgt = ldpool.tile([128, gch * 128], BF16, name=f"sg{lnum}", tag="sg")
                    nc.sync.dma_start(out=sgt[:, :nch * 128], in_=sg_in[g, :, :nch * 128])
                    for tl in range(ntl):
                        t = g * tpg + tl
                        if lnum == 1:
                            po = pbig.tile([128, feat], F32, name="po1", tag="acc")
                            pd = pden.tile([128, nheads], F32, name="pd1", tag="den")
                            pt = wpool.tile([128, nheads], BF16, name="pt1")
                        else:
                            po = pbig.tile([128, feat + 1], F32, name=f"po{lnum}", tag="acc")
                        for ch in range(cpt):
                            lc = tl * cpt + ch
                            first, last = ch == 0, ch == cpt - 1
                            gt = gatpool.tile([128, row], BF16, name=f"g{lnum}",
                                              tag=f"g{lnum}")
                            nc.gpsimd.indirect_dma_start(
                                out=gt[:, :],
                                out_offset=None,
                                in_=table_full[:, :],
                                in_offset=bass.IndirectOffsetOnAxis(
                                    ap=idxt[:, lc:lc + 1], axis=0),
                            )
                            S = sgt[:, lc * 128:(lc + 1) * 128]
                            pE = ppe.tile([128, nheads], F32, name=f"pE{lnum}", tag="pE")
                            if lnum == 1:
                                edt = esd1_keep[:, t * 16 + 8:t * 16 + 16]
                            elif lnum == 2:
                                edt = esd2_keep[:, t * 2 + 1:t * 2 + 2]
                            else:
                                edt = esd3_keep[:, t * 2 + 1:t * 2 + 2]
                            nc.tensor.matmul(
                                pE[:], lhsT=stt[:, lc * 128:(lc + 1) * 128],
                                rhs=edt, start=True, stop=True)
                            Ee = wpool.tile([128, nheads], F32, name=f"Ee{lnum}", tag="Ee")
                            nc.vector.tensor_tensor(
                                out=Ee[:], in0=gt[:, feat:feat + nheads],
                                in1=pE[:], op=ALU.add)
                            lr = wpool.tile([128, nheads], F32, name=f"lr{lnum}", tag="lr")
                            nc.vector.scalar_tensor_tensor(
                                out=lr[:], in0=Ee[:], scalar=NEG, in1=Ee[:],
                                op0=ALU.mult, op1=ALU.max)
                            if lnum == 1:
                                nc.scalar.activation(out=pt[:], in_=lr[:], func=AF.Exp)
                                Mw = wpool.tile([128, feat], BF16, name="Mw1")
                                nc.vector.tensor_tensor(
                                    out=Mw[:].rearrange("p (h c) -> p h c", h=HEADS),
                                    in0=gt[:, 0:feat].rearrange(
                                        "p (h c) -> p h c", h=HEADS),
                                    in1=pt[:, :, None].to_broadcast([128, HEADS, HID]),
                                    op=ALU.mult)
                                nc.tensor.matmul(po[:], lhsT=S, rhs=Mw[:],
                                                 start=first, stop=last)
                                nc.tensor.matmul(pd[:], lhsT=S, rhs=pt[:],
                                                 start=first, stop=last)
                            else:
                                Mw = wpool.tile([128, feat + 1], BF16, name=f"Mw{lnum}", tag="Mw23")
                                ptf = wpool.tile([128, 1], F32, name=f"ptf{lnum}", tag="ptf")
                                nc.scalar.activation(
                                    out=ptf[:], in_=lr[:], func=AF.Exp)
                                nc.scalar.copy(
                                    out=Mw[:, feat:feat + 1], in_=ptf[:])
                                nc.vector.tensor_scalar(
                                    out=Mw[:, 0:feat], in0=gt[:, 0:feat],
                                    scalar1=ptf[:], scalar2=None,
                                    op0=ALU.mult)
                                nc.tensor.matmul(po[:], lhsT=S, rhs=Mw[:],
                                                 start=first, stop=last)
                        if lnum == 1:
                            finalize(t, po, pd)
                        else:
                            finalize(t, po, None)

            # ---------------- Layer 1 ----------------
            def fin1(t, po, pd):
                # self-loop (local, no gather): p_self*h_tile added to po/pd
                hs = fpool.tile([128, HEADS * HID], BF16, name="hs1")
                nc.sync.dma_start(
                    out=hs[:], in_=t1_loc[t * 128:(t + 1) * 128, 0:HEADS * HID])
                Es = fpool.tile([128, HEADS], F32, name="Es1")
                nc.vector.tensor_tensor(
                    out=Es[:], in0=esd1_keep[:, t * 16:t * 16 + 8],
                    in1=esd1_keep[:, t * 16 + 8:t * 16 + 16], op=ALU.add)
                lrs = fpool.tile([128, HEADS], F32, name="lrs1")
                nc.vector.scalar_tensor_tensor(
                    out=lrs[:], in0=Es[:], scalar=NEG, in1=Es[:],
                    op0=ALU.mult, op1=ALU.max)
                psf = fpool.tile([128, HEADS], F32, name="psf1")
                nc.scalar.activation(out=psf[:], in_=lrs[:], func=AF.Exp)
                den = fpool.tile([128, HEADS], F32, name="den1")
                nc.vector.tensor_tensor(
                    out=den[:], in0=pd[:], in1=psf[:], op=ALU.add)
                rec = fpool.tile([128, HEADS], F32, name="rec1")
                nc.vector.reciprocal(rec[:], den[:])
                acc = fpool.tile([128, HEADS * HID], F32, name="acc1")
                nc.vector.tensor_tensor(
                    out=acc[:].rearrange("p (h c) -> p h c", h=HEADS),
                    in0=hs[:].rearrange("p (h c) -> p h c", h=HEADS),
                    in1=psf[:, :, None].to_broadcast([128, HEADS, HID]),
                    op=ALU.mult)
                nc.vector.tensor_tensor(
                    out=acc[:], in0=acc[:], in1=po[:], op=ALU.add)
                x1f = fpool.tile([128, HEADS * HID], F32, name="x1f")
                nc.vector.tensor_tensor(
                    out=x1f[:].rearrange("p (h c) -> p h c", h=HEADS),
                    in0=acc[:].rearrange("p (h c) -> p h c", h=HEADS),
                    in1=rec[:, :, None].to_broadcast([128, HEADS, HID]),
                    op=ALU.mult)
                x1t = fpool.tile([128, HEADS * HID], BF16, name="x1t")
                nc.scalar.activation(out=x1t[:], in_=x1f[:], func=AF.Relu)
                # layer-2 table rows
                x1T = fpool.tile([128, HEADS * HID], BF16, name="x1T")
                for k in range(4):
                    tp = pmid.tile([128, 128], BF16, name="tpT", tag="mid")
                    nc.tensor.transpose(
                        out=tp[:], in_=x1t[:, k * 128:(k + 1) * 128],
                        identity=ident[:])
                    nc.scalar.copy(out=x1T[:, k * 128:(k + 1) * 128], in_=tp[:])
                pt2 = pmid.tile([128, ROW2], F32, name="pt2", tag="mid")
                for k in range(4):
                    nc.tensor.matmul(
                        pt2[:], lhsT=x1T[:, k * 128:(k + 1) * 128],
                        rhs=w2sd[:, k * ROW2:(k + 1) * ROW2],
                        start=(k == 0), stop=(k == 3))
                t2t = fpool.tile([128, ROW2], BF16, name="t2t")
                nc.vector.tensor_tensor(
                    out=t2t[:, 0:HID], in0=pt2[:, 0:HID], in1=b2b[:], op=ALU.add)
                nc.scalar.copy(out=t2t[:, HID:ROW2], in_=pt2[:, HID:ROW2])
                nc.vector.tensor_copy(
                    out=esd2_keep[:, t * 2:(t + 1) * 2], in_=t2t[:, HID:ROW2])
                nc.sync.dma_start(out=t2_loc[t * 128:(t + 1) * 128, :], in_=t2t[:])
                if t % ag_t == ag_t - 1:
                    allgather_chunk(t2_loc, t2_full, t // ag_t)

            agg_layer(1, t1_full, ROW1, HEADS * HID, HEADS, fin1)

            # ---------------- Layer 2 ----------------
            def fin2(t, po, _):
                hs = fpool.tile([128, HID], BF16, name="hs2")
                nc.sync.dma_start(
                    out=hs[:], in_=t2_loc[t * 128:(t + 1) * 128, 0:HID])
                Es = fpool.tile([128, 1], F32, name="Es2")
                nc.vector.tensor_tensor(
                    out=Es[:], in0=esd2_keep[:, t * 2:t * 2 + 1],
                    in1=esd2_keep[:, t * 2 + 1:t * 2 + 2], op=ALU.add)
                lrs = fpool.tile([128, 1], F32, name="lrs2")
                nc.vector.scalar_tensor_tensor(
                    out=lrs[:], in0=Es[:], scalar=NEG, in1=Es[:],
                    op0=ALU.mult, op1=ALU.max)
                psf = fpool.tile([128, 1], F32, name="psf2")
                nc.scalar.activation(out=psf[:], in_=lrs[:], func=AF.Exp)
                den = fpool.tile([128, 1], F32, name="den2")
                nc.vector.tensor_tensor(
                    out=den[:], in0=po[:, HID:HID + 1], in1=psf[:], op=ALU.add)
                rec = fpool.tile([128, 1], F32, name="rec2")
                nc.vector.reciprocal(rec[:], den[:])
                acc = fpool.tile([128, HID], F32, name="acc2")
                nc.vector.tensor_scalar(
                    out=acc[:], in0=hs[:], scalar1=psf[:], scalar2=None,
                    op0=ALU.mult)
                nc.vector.tensor_tensor(
                    out=acc[:], in0=acc[:], in1=po[:, 0:HID], op=ALU.add)
                x2f = fpool.tile([128, HID], F32, name="x2f")
                nc.vector.tensor_scalar(
                    out=x2f[:], in0=acc[:], scalar1=rec[:, 0:1],
                    scalar2=None, op0=ALU.mult)
                x2t = fpool.tile([128, HID], BF16, name="x2t")
                nc.scalar.activation(out=x2t[:], in_=x2f[:], func=AF.Relu)
                tp = pmid.tile([128, 128], BF16, name="tpT2", tag="mid")
                nc.tensor.transpose(out=tp[:HID, :], in_=x2t[:], identity=ident[:])
                x2T = fpool.tile([HID, 128], BF16, name="x2T")
                nc.scalar.copy(out=x2T[:], in_=tp[:HID, :])
                pt3 = pmid.tile([128, OUT + 2], F32, name="pt3", tag="mid")
                nc.tensor.matmul(pt3[:], lhsT=x2T[:], rhs=w3sd[:],
                                 start=True, stop=True)
                t3t = fpool.tile([128, ROW3], BF16, name="t3t")
                nc.vector.tensor_tensor(
                    out=t3t[:, 0:OUT], in0=pt3[:, 0:OUT], in1=b3b[:], op=ALU.add)
                nc.scalar.copy(out=t3t[:, OUT:OUT + 2], in_=pt3[:, OUT:OUT + 2])
                nc.vector.memset(t3t[:, OUT + 2:ROW3], 0)
                nc.vector.tensor_copy(
                    out=esd3_keep[:, t * 2:(t + 1) * 2], in_=t3t[:, OUT:OUT + 2])
                nc.sync.dma_start(out=t3_loc[t * 128:(t + 1) * 128, :], in_=t3t[:])
                if t % ag_t == ag_t - 1:
                    allgather_chunk(t3_loc, t3_full, t // ag_t)

            agg_layer(2, t2_full, ROW2, HID, 1, fin2)

            # ---------------- Layer 3 ----------------
            def fin3(t, po, _):
                hs = fpool.tile([128, OUT], BF16, name="hs3")
                nc.sync.dma_start(
                    out=hs[:], in_=t3_loc[t * 128:(t + 1) * 128, 0:OUT])
                Es = fpool.tile([128, 1], F32, name="Es3")
                nc.vector.tensor_tensor(
                    out=Es[:], in0=esd3_keep[:, t * 2:t * 2 + 1],
                    in1=esd3_keep[:, t * 2 + 1:t * 2 + 2], op=ALU.add)
                lrs = fpool.tile([128, 1], F32, name="lrs3")
                nc.vector.scalar_tensor_tensor(
                    out=lrs[:], in0=Es[:], scalar=NEG, in1=Es[:],
                    op0=ALU.mult, op1=ALU.max)
                psf = fpool.tile([128, 1], F32, name="psf3")
                nc.scalar.activation(out=psf[:], in_=lrs[:], func=AF.Exp)
                den = fpool.tile([128, 1], F32, name="den3")
                nc.vector.tensor_tensor(
                    out=den[:], in0=po[:, OUT:OUT + 1], in1=psf[:], op=ALU.add)
                rec = fpool.tile([128, 1], F32, name="rec3")
                nc.vector.reciprocal(rec[:], den[:])
                acc = fpool.tile([128, OUT], F32, name="acc3")
                nc.vector.tensor_scalar(
                    out=acc[:], in0=hs[:], scalar1=psf[:], scalar2=None,
                    op0=ALU.mult)
                nc.vector.tensor_tensor(
                    out=acc[:], in0=acc[:], in1=po[:, 0:OUT], op=ALU.add)
                of = fpool.tile([128, OUT], F32, name="of")
                nc.vector.tensor_scalar(
                    out=of[:], in0=acc[:], scalar1=rec[:, 0:1],
                    scalar2=None, op0=ALU.mult)
                nc.sync.dma_start(out=out_ext[t * 128:(t + 1) * 128, :], in_=of[:])

            agg_layer(3, t3_full, ROW3, OUT, 1, fin3)

        for _f in reversed(_frees):
            _f()

    nc.compile()
    return nc


# ----------------------------------------------------------------------------
# Entry points
# ----------------------------------------------------------------------------

def _install_trace_hook():
    """Provide antenv.axon_hooks (missing in this image) so that
    run_bass_kernel_spmd(trace=True) can reach the axon NTFF profiler."""
    if "antenv.axon_hooks" in sys.modules:
        return
    import types
    import ctypes
    import contextlib

    so_path = "/opt/axon/libaxon_pjrt.so"
    mod = types.ModuleType("antenv.axon_hooks")
    state = {}

    def set_axon_ntff_profile_hook(h):
        state["h"] = h

    def get_axon_ntff_profile_hook():
        return state.get("h")

    mod.set_axon_ntff_profile_hook = set_axon_ntff_profile_hook
    mod.get_axon_ntff_profile_hook = get_axon_ntff_profile_hook
    sys.modules["antenv.axon_hooks"] = mod
    try:
        import antenv

        antenv.axon_hooks = mod
    except ImportError:
        pass

    try:
        lib = ctypes.CDLL(so_path)
    except OSError:
        return
    if not hasattr(lib, "axon_start_nrt_profile"):
        return
    lib.axon_start_nrt_profile.argtypes = [
        ctypes.POINTER(ctypes.c_int64), ctypes.c_size_t]
    lib.axon_start_nrt_profile.restype = ctypes.c_int64
    lib.axon_stop_nrt_profile.argtypes = [ctypes.c_char_p]
    lib.axon_stop_nrt_profile.restype = ctypes.c_int64

    @contextlib.contextmanager
    def _hook(output_dir, device_ids):
        import jax

        jax.devices()
        if device_ids:
            ids = (ctypes.c_int64 * len(device_ids))(*device_ids)
            rc = lib.axon_start_nrt_profile(ids, len(device_ids))
        else:
            rc = lib.axon_start_nrt_profile(None, 0)
        if rc != 0:
            raise RuntimeError(f"axon_start_nrt_profile rc={rc}")
        try:
            yield
        finally:
            n = lib.axon_stop_nrt_profile(str(output_dir).encode())
            if n < 0:
                raise RuntimeError(f"axon_stop_nrt_profile rc={n}")

    set_axon_ntff_profile_hook(_hook)


def _run(inputs, n_cores=8, t_pc=49, tpg=4, trace=False):
    global LAST_EXEC_NS
    if trace:
        _install_trace_hook()
    prep = _prepare(inputs, n_cores, t_pc, tpg)
    nc = _build(n_cores, t_pc, prep["cpt"], tpg)
    in_maps = prep["per_core"]
    res = run_bass_kernel_spmd(nc, in_maps, list(range(n_cores)), trace=trace)
    LAST_EXEC_NS = res.exec_time_ns
    outs = np.concatenate([res.results[c]["out"] for c in range(n_cores)], 0)
    n = np.asarray(inputs["x"]).shape[0]
    return np.ascontiguousarray(outs[prep["pos"][:n]]).astype(np.float32)


def kernel(**inputs):
    trace = os.environ.get("GAT_TRACE", "0") == "1"
    return _run(inputs, trace=trace)
